# revision 4
# baseline (speedup 1.0000x reference)
"""Trainium2 Bass kernel for nn_MemoryAwareAKTAttention.

Math (per batch b):
    integrated = concat([x, mem], -1) @ Wm.T + bm          [S, E]
    q, k, v    = heads(integrated @ W{q,k,v}.T)            [H, S, D]
    scores     = q @ k.T / sqrt(D)                         [H, S, S]
    decay      = exp(-softplus(gamma_h) * |i-j|)           [H, S, S]
    total      = clip(exp(scores * decay), 1e-5, 1e5)
    attn       = total / (sum_j total + 1e-9)
    out        = (attn @ v) @ Wo.T + bo                    [S, E]

Structure exploited: with g = softplus(gamma) >= ~0.17, decay underflows to
0.0 within ~129 columns of the diagonal, so total == 1.0f bit-exactly outside
a narrow band (exp(x) rounds to 1.0f for |x| < 2^-25).  Per 128-row query
block we compute an exact W-wide window around the diagonal; off-window
attn[i, j] == 1/rowsum[i] is a per-row constant fill, and
out = diag(r) @ ((total-1)_win @ v + ones @ v) folds the off-band matmul into
a per-head column-sum.

Sharding: data-parallel over batch, one batch element per NeuronCore (B=8).
"""

import os
import numpy as np

import concourse.bass as bass
import concourse.mybir as mybir
import concourse.tile as tile
from concourse import bacc
from concourse.bass_utils import run_bass_kernel_spmd
from concourse.masks import make_identity

F32 = mybir.dt.float32
AF = mybir.ActivationFunctionType
ALU = mybir.AluOpType

B, S, E = 8, 1024, 512
H = 8
D = E // H          # 64
TWO_E = 2 * E
NCORES = 8
SQ = S // 128       # query blocks per core

LAST_PERF = {}      # filled by kernel(): exec_time_ns etc.
_CACHE = {}         # W -> compiled Bacc module


def _wstart(qb: int, W: int) -> int:
    """Window start for query block qb; multiple of 128, centered, clipped."""
    n = W // 128
    return min(max((qb - (n - 1) // 2) * 128, 0), S - W)


def _pick_window(g_min: float) -> int:
    """Smallest window width such that off-window total == 1.0f exactly.

    Off-window distance >= margin(W); need exp(-g*margin) * SMAX < 2^-25
    with SMAX = 64 a generous bound on |scores|.  ln(64 * 2^26) ~= 22.18.
    """
    for n in (3, 5, 7):
        W = n * 128
        margin = ((n - 1) // 2 - 1) * 128 + 129
        if g_min * margin >= 22.18:
            return W
    return S  # dense fallback: every column computed exactly


def _build(W: int) -> bacc.Bacc:
    nc = bacc.Bacc(None)
    NW = W // 128

    combT_d = nc.dram_tensor("combT", [TWO_E, S], F32, kind="ExternalInput")
    WmT_d = nc.dram_tensor("WmT", [TWO_E, E], F32, kind="ExternalInput")
    WqT_d = nc.dram_tensor("WqT", [E, E], F32, kind="ExternalInput")
    WkT_d = nc.dram_tensor("WkT", [E, E], F32, kind="ExternalInput")
    WvT_d = nc.dram_tensor("WvT", [E, E], F32, kind="ExternalInput")
    WoT_d = nc.dram_tensor("WoT", [E, E], F32, kind="ExternalInput")
    bm_d = nc.dram_tensor("bm", [E], F32, kind="ExternalInput")
    bo_d = nc.dram_tensor("bo", [E], F32, kind="ExternalInput")
    gam_d = nc.dram_tensor("gam", [H], F32, kind="ExternalInput")
    out_d = nc.dram_tensor("out", [S, E], F32, kind="ExternalOutput")
    attn_d = nc.dram_tensor("attn", [H, S, S], F32, kind="ExternalOutput")

    with tile.TileContext(nc) as tc:
        with (
            tc.tile_pool(name="const", bufs=1) as const,
            tc.tile_pool(name="persist", bufs=1) as persist,
        ):
            # ---- constants -------------------------------------------------
            ident = const.tile([128, 128], F32)
            make_identity(nc, ident[:])
            ones_row = const.tile([1, 128], F32)
            nc.vector.memset(ones_row[:], 1.0)
            ones_col = const.tile([128, 1], F32)
            nc.vector.memset(ones_col[:], 1.0)
            ones_fl = const.tile([128, S], F32)
            nc.vector.memset(ones_fl[:], 1.0)

            gam_sb = const.tile([1, H], F32)
            nc.gpsimd.dma_start(out=gam_sb[:], in_=gam_d[None, :])
            # softplus(x) = ln(1 + exp(x)); Softplus has no ACT table here
            g_e = const.tile([1, H], F32)
            nc.scalar.activation(g_e[:], gam_sb[:], AF.Exp)
            g_e1 = const.tile([1, H], F32)
            nc.vector.tensor_scalar_add(g_e1[:], g_e[:], 1.0)
            g_sp = const.tile([1, H], F32)
            nc.scalar.activation(g_sp[:], g_e1[:], AF.Ln)
            ng = const.tile([1, H], F32)
            nc.vector.tensor_scalar_mul(ng[:], g_sp[:], -1.0)

            bmT = const.tile([128, E // 128], F32)
            nc.gpsimd.dma_start(
                out=bmT[:], in_=bm_d.rearrange("(m p) -> p m", p=128)
            )
            bo_sb = const.tile([1, E], F32)
            nc.gpsimd.dma_start(out=bo_sb[:], in_=bo_d[None, :])

            # persistent activations
            qT_sb = persist.tile([128, 4, S], F32)   # q^T / 8, [e, s] layout
            kT_sb = persist.tile([128, 4, S], F32)   # k^T
            v_sb = persist.tile([128, SQ, E], F32)   # v natural [s, e]
            outT_sb = persist.tile([128, 4, S], F32)  # per-head outputs^T
            WoT_sb = persist.tile([128, 4, E], F32)
            nc.sync.dma_start(
                out=WoT_sb[:], in_=WoT_d.rearrange("(c p) e -> p c e", p=128)
            )
            ngbc = persist.tile([128, H], F32)
            colsum_sb = persist.tile([1, E], F32)

            # ---- phase 1+2: projections -----------------------------------
            with (
                tc.tile_pool(name="ph12", bufs=1) as ph12,
                tc.tile_pool(name="ps12", bufs=3, space="PSUM") as ps12,
                tc.tile_pool(name="ps12s", bufs=2, space="PSUM") as ps12s,
            ):
                combT_sb = ph12.tile([128, 8, S], F32)
                nc.sync.dma_start(
                    out=combT_sb[:],
                    in_=combT_d.rearrange("(c p) s -> p c s", p=128),
                )
                WmT_sb = ph12.tile([128, 8, E], F32)
                nc.sync.dma_start(
                    out=WmT_sb[:], in_=WmT_d.rearrange("(c p) e -> p c e", p=128)
                )
                WqT_sb = ph12.tile([128, 4, E], F32)
                nc.sync.dma_start(
                    out=WqT_sb[:], in_=WqT_d.rearrange("(c p) e -> p c e", p=128)
                )
                WkT_sb = ph12.tile([128, 4, E], F32)
                nc.sync.dma_start(
                    out=WkT_sb[:], in_=WkT_d.rearrange("(c p) e -> p c e", p=128)
                )
                WvT_sb = ph12.tile([128, 4, E], F32)
                nc.sync.dma_start(
                    out=WvT_sb[:], in_=WvT_d.rearrange("(c p) e -> p c e", p=128)
                )

                # -g broadcast across partitions via K=1 matmul
                ps_nb = ps12s.tile([128, H], F32, tag="s")
                nc.tensor.matmul(
                    ps_nb[:], ones_row[:], ng[:], start=True, stop=True
                )
                nc.scalar.copy(ngbc[:], ps_nb[:])

                # integrated^T [e, s]
                intT_sb = ph12.tile([128, 4, S], F32)
                for m in range(4):
                    for sh in range(2):
                        ps_i = ps12.tile([128, 512], F32, tag="mm")
                        for k in range(8):
                            nc.tensor.matmul(
                                ps_i[:],
                                WmT_sb[:, k, m * 128:(m + 1) * 128],
                                combT_sb[:, k, sh * 512:(sh + 1) * 512],
                                start=(k == 0),
                                stop=(k == 7),
                            )
                        nc.scalar.activation(
                            intT_sb[:, m, sh * 512:(sh + 1) * 512],
                            ps_i[:],
                            AF.Identity,
                            bias=bmT[:, m:m + 1],
                        )

                # q^T (scaled by 1/sqrt(D)), k^T
                for (w_sb, dst, scl) in (
                    (WqT_sb, qT_sb, 1.0 / np.sqrt(D)),
                    (WkT_sb, kT_sb, 1.0),
                ):
                    for m in range(4):
                        for sh in range(2):
                            ps_q = ps12.tile([128, 512], F32, tag="mm")
                            for k in range(4):
                                nc.tensor.matmul(
                                    ps_q[:],
                                    w_sb[:, k, m * 128:(m + 1) * 128],
                                    intT_sb[:, k, sh * 512:(sh + 1) * 512],
                                    start=(k == 0),
                                    stop=(k == 3),
                                )
                            if scl != 1.0:
                                nc.scalar.mul(
                                    dst[:, m, sh * 512:(sh + 1) * 512],
                                    ps_q[:], scl,
                                )
                            else:
                                nc.scalar.copy(
                                    dst[:, m, sh * 512:(sh + 1) * 512], ps_q[:]
                                )

                # v natural [s, e]
                for sc in range(SQ):
                    ps_v = ps12.tile([128, 512], F32, tag="mm")
                    for k in range(4):
                        nc.tensor.matmul(
                            ps_v[:],
                            intT_sb[:, k, sc * 128:(sc + 1) * 128],
                            WvT_sb[:, k, :],
                            start=(k == 0),
                            stop=(k == 3),
                        )
                    nc.scalar.copy(v_sb[:, sc, :], ps_v[:])

                # column sums of v (all heads at once): [1, E]
                ps_cs = ps12s.tile([1, E], F32, tag="s")
                for sc in range(SQ):
                    nc.tensor.matmul(
                        ps_cs[:],
                        ones_col[:],
                        v_sb[:, sc, :],
                        start=(sc == 0),
                        stop=(sc == SQ - 1),
                    )
                nc.scalar.copy(colsum_sb[:], ps_cs[:])

            # ---- phase 3: banded attention --------------------------------
            sb_bufs = 2 if W <= 640 else 1
            with (
                tc.tile_pool(name="work", bufs=2) as work,
                tc.tile_pool(name="tot_p", bufs=3) as tot_p,
                tc.tile_pool(name="attn_p", bufs=3) as attn_p,
                tc.tile_pool(name="ps_s", bufs=sb_bufs, space="PSUM") as ps_s,
                tc.tile_pool(name="ps_t", bufs=sb_bufs, space="PSUM") as ps_t,
                tc.tile_pool(name="ps_o", bufs=2, space="PSUM") as ps_o,
                tc.tile_pool(name="ps_r", bufs=2, space="PSUM") as ps_r,
            ):
                for qb in range(SQ):
                    ws = _wstart(qb, W)
                    # dist tile: |i - j| for i in this block, j in window
                    dist_i = work.tile([128, W], F32, tag="dist_i")
                    nc.gpsimd.iota(
                        dist_i[:],
                        pattern=[[-1, W]],
                        base=qb * 128 - ws,
                        channel_multiplier=1,
                        allow_small_or_imprecise_dtypes=True,
                    )
                    dist_a = work.tile([128, W], F32, tag="dist_a")
                    nc.scalar.activation(dist_a[:], dist_i[:], AF.Abs)

                    for h in range(8):
                        po, mc = (h % 2) * 64, h // 2
                        # decay = exp(-g_h * dist)
                        decay = work.tile([128, W], F32, tag="decay")
                        nc.scalar.activation(
                            decay[:], dist_a[:], AF.Exp, scale=ngbc[:, h:h + 1]
                        )
                        # scores window
                        ps_sc = ps_s.tile([128, W], F32)
                        for j0 in range(0, W, 512):
                            jw = min(512, W - j0)
                            nc.tensor.matmul(
                                ps_sc[:, j0:j0 + jw],
                                qT_sb[po:po + 64, mc, qb * 128:(qb + 1) * 128],
                                kT_sb[po:po + 64, mc, ws + j0:ws + j0 + jw],
                                start=True,
                                stop=True,
                            )
                        # sd = scores * decay;  total = exp(sd), rowsum
                        sd = work.tile([128, W], F32, tag="sd")
                        nc.vector.tensor_mul(sd[:], ps_sc[:], decay[:])
                        tot = tot_p.tile([128, W], F32, tag="tot")
                        rs = work.tile([128, 1], F32, tag="rs")
                        nc.scalar.activation(
                            tot[:], sd[:], AF.Exp, accum_out=rs[:]
                        )
                        rs2 = work.tile([128, 1], F32, tag="rs2")
                        nc.vector.tensor_scalar_add(rs2[:], rs[:], float(S - W))
                        r_t = work.tile([128, 1], F32, tag="r_t")
                        nc.vector.reciprocal(r_t[:], rs2[:])

                        # attn tile: off-window fill r, window total*r
                        at = attn_p.tile([128, S], F32, tag="attn")
                        if ws > 0:
                            nc.vector.tensor_scalar_mul(
                                at[:, 0:ws], ones_fl[:, 0:ws], r_t[:]
                            )
                        if ws + W < S:
                            nc.vector.tensor_scalar_mul(
                                at[:, ws + W:S],
                                ones_fl[:, 0:S - ws - W],
                                r_t[:],
                            )
                        nc.vector.tensor_scalar_mul(
                            at[:, ws:ws + W], tot[:], r_t[:]
                        )
                        nc.sync.dma_start(
                            out=attn_d[h, qb * 128:(qb + 1) * 128, :],
                            in_=at[:],
                        )

                        # rtb1 = (total - 1) * r  (row-scaled band)
                        rtb1 = work.tile([128, W], F32, tag="rtb1")
                        nc.vector.tensor_scalar(
                            rtb1[:], tot[:], 1.0, r_t[:],
                            ALU.subtract, ALU.mult,
                        )
                        # r^T as a row [1, 128]
                        ps_rt = ps_r.tile([1, 128], F32)
                        nc.tensor.matmul(
                            ps_rt[:], r_t[:], ident[:], start=True, stop=True
                        )
                        rT_sb = work.tile([1, 128], F32, tag="rT")
                        nc.scalar.copy(rT_sb[:], ps_rt[:])

                        # transpose band chunks
                        ps_tr = ps_t.tile([128, W], F32)
                        for c in range(NW):
                            nc.tensor.transpose(
                                ps_tr[:, c * 128:(c + 1) * 128],
                                rtb1[:, c * 128:(c + 1) * 128],
                                ident[:],
                            )
                        rtb1T = work.tile([128, NW, 128], F32, tag="rtb1T")
                        nc.scalar.copy(rtb1T[:], ps_tr[:])

                        # out^T_h[d, q] = v^T @ rtb1T + colsum ⊗ r^T
                        ps_ov = ps_o.tile([64, 128], F32)
                        for c in range(NW):
                            tci = ws // 128 + c
                            nc.tensor.matmul(
                                ps_ov[:],
                                v_sb[:, tci, h * 64:(h + 1) * 64],
                                rtb1T[:, c, :],
                                start=(c == 0),
                                stop=False,
                            )
                        nc.tensor.matmul(
                            ps_ov[:],
                            colsum_sb[0:1, h * 64:(h + 1) * 64],
                            rT_sb[:],
                            start=False,
                            stop=True,
                        )
                        nc.scalar.copy(
                            outT_sb[po:po + 64, mc, qb * 128:(qb + 1) * 128],
                            ps_ov[:],
                        )

            # ---- phase 4: output projection -------------------------------
            with (
                tc.tile_pool(name="ph4", bufs=3) as ph4,
                tc.tile_pool(name="ps4", bufs=2, space="PSUM") as ps4,
            ):
                for qb in range(SQ):
                    ps_f = ps4.tile([128, E], F32)
                    for ec in range(4):
                        nc.tensor.matmul(
                            ps_f[:],
                            outT_sb[:, ec, qb * 128:(qb + 1) * 128],
                            WoT_sb[:, ec, :],
                            start=(ec == 0),
                            stop=False,
                        )
                    nc.tensor.matmul(
                        ps_f[:], ones_row[:], bo_sb[:], start=False, stop=True
                    )
                    o_sb = ph4.tile([128, E], F32, tag="o")
                    nc.scalar.copy(o_sb[:], ps_f[:])
                    nc.sync.dma_start(
                        out=out_d[qb * 128:(qb + 1) * 128, :], in_=o_sb[:]
                    )

    nc.compile()
    return nc


def kernel(**inputs):
    x = np.asarray(inputs["unified_embed"], dtype=np.float32)
    mem = np.asarray(inputs["memory_state"], dtype=np.float32)
    Wq = np.asarray(inputs["Wq"], dtype=np.float32)
    Wk = np.asarray(inputs["Wk"], dtype=np.float32)
    Wv = np.asarray(inputs["Wv"], dtype=np.float32)
    Wm = np.asarray(inputs["Wm"], dtype=np.float32)
    bm = np.asarray(inputs["bm"], dtype=np.float32)
    Wo = np.asarray(inputs["Wo"], dtype=np.float32)
    bo = np.asarray(inputs["bo"], dtype=np.float32)
    gammas = np.asarray(inputs["gammas"], dtype=np.float32)

    g = np.logaddexp(0.0, gammas.astype(np.float64))  # softplus, host-side
    W = _pick_window(float(g.min()))

    if W not in _CACHE:
        _CACHE[W] = _build(W)
    nc = _CACHE[W]

    WmT = np.ascontiguousarray(Wm.T)
    WqT = np.ascontiguousarray(Wq.T)
    WkT = np.ascontiguousarray(Wk.T)
    WvT = np.ascontiguousarray(Wv.T)
    WoT = np.ascontiguousarray(Wo.T)

    in_maps = []
    for b in range(NCORES):
        combT = np.ascontiguousarray(
            np.concatenate([x[b], mem[b]], axis=1).T
        )
        in_maps.append({
            "combT": combT,
            "WmT": WmT, "WqT": WqT, "WkT": WkT, "WvT": WvT, "WoT": WoT,
            "bm": bm, "bo": bo, "gam": gammas,
        })

    res = run_bass_kernel_spmd(
        nc, in_maps, core_ids=list(range(NCORES)),
        trace=bool(int(os.environ.get("KERNEL_TRACE", "0"))),
    )
    LAST_PERF["exec_time_ns"] = res.exec_time_ns
    LAST_PERF["mean_exec_time_ns"] = res.mean_exec_time_ns
    LAST_PERF["trace"] = res.instructions_and_trace

    out = np.stack([res.results[b]["out"] for b in range(NCORES)])
    attn = np.stack([res.results[b]["attn"] for b in range(NCORES)])
    return out, attn


# revision 7
# speedup vs baseline: 1.3433x; 1.3433x over previous
"""Trainium2 Bass kernel for nn_MemoryAwareAKTAttention.

Math (per batch b):
    integrated = concat([x, mem], -1) @ Wm.T + bm          [S, E]
    q, k, v    = heads(integrated @ W{q,k,v}.T)            [H, S, D]
    scores     = q @ k.T / sqrt(D)                         [H, S, S]
    decay      = exp(-softplus(gamma_h) * |i-j|)           [H, S, S]
    total      = clip(exp(scores * decay), 1e-5, 1e5)
    attn       = total / (sum_j total + 1e-9)
    out        = (attn @ v) @ Wo.T + bo                    [S, E]

Structure exploited: with g = softplus(gamma) >= ~0.17, decay underflows to
0.0 within ~129 columns of the diagonal, so total == 1.0f bit-exactly outside
a narrow band (exp(x) rounds to 1.0f for |x| < 2^-25).  Per 128-row query
block we compute an exact W-wide window around the diagonal; off-window
attn[i, j] == 1/rowsum[i] is a per-row constant fill, and
out = diag(r) @ ((total-1)_win @ v + ones @ v) folds the off-band matmul into
a per-head column-sum (a rank-1 colsum x r^T matmul seeds the accumulator).

Matmuls run in float32r (full PE rate vs 4 cycles/row for fp32); the
exp/normalization chain stays fp32.

Sharding: data-parallel over batch, one batch element per NeuronCore (B=8).
"""

import os
import numpy as np

import concourse.bass as bass
import concourse.mybir as mybir
import concourse.tile as tile
from concourse import bacc
from concourse.bass_utils import run_bass_kernel_spmd
from concourse.masks import make_identity

F32 = mybir.dt.float32
F32R = mybir.dt.float32r
AF = mybir.ActivationFunctionType
ALU = mybir.AluOpType

B, S, E = 8, 1024, 512
H = 8
D = E // H          # 64
TWO_E = 2 * E
NCORES = 8
SQ = S // 128       # query blocks per core

LAST_PERF = {}      # filled by kernel(): exec_time_ns etc.
_CACHE = {}         # W -> compiled Bacc module


def _wstart(qb: int, W: int) -> int:
    """Window start for query block qb; multiple of 128, centered, clipped."""
    n = W // 128
    return min(max((qb - (n - 1) // 2) * 128, 0), S - W)


def _pick_window(g_min: float) -> int:
    """Smallest window width such that off-window total == 1.0f exactly.

    Off-window distance >= margin(W); need exp(-g*margin) * SMAX < 2^-25
    with SMAX = 64 a generous bound on |scores|.  ln(64 * 2^26) ~= 22.18.
    """
    for n in (3, 5, 7):
        W = n * 128
        margin = ((n - 1) // 2 - 1) * 128 + 129
        if g_min * margin >= 22.18:
            return W
    return S  # dense fallback: every column computed exactly


def _build(W: int) -> bacc.Bacc:
    nc = bacc.Bacc(None)
    NW = W // 128
    half = (NW - 1) // 2  # window chunks on each side of the diagonal chunk

    combT_d = nc.dram_tensor("combT", [TWO_E, S], F32R, kind="ExternalInput")
    WmT_d = nc.dram_tensor("WmT", [TWO_E, E], F32R, kind="ExternalInput")
    WqT_d = nc.dram_tensor("WqT", [E, E], F32R, kind="ExternalInput")
    WkT_d = nc.dram_tensor("WkT", [E, E], F32R, kind="ExternalInput")
    WvT_d = nc.dram_tensor("WvT", [E, E], F32R, kind="ExternalInput")
    WoT_d = nc.dram_tensor("WoT", [E, E], F32R, kind="ExternalInput")
    bm_d = nc.dram_tensor("bm", [E], F32, kind="ExternalInput")
    bo_d = nc.dram_tensor("bo", [E], F32R, kind="ExternalInput")
    gam_d = nc.dram_tensor("gam", [H], F32, kind="ExternalInput")
    out_d = nc.dram_tensor("out", [S, E], F32, kind="ExternalOutput")
    attn_d = nc.dram_tensor("attn", [H, S, S], F32, kind="ExternalOutput")

    with tile.TileContext(nc) as tc:
        with (
            tc.tile_pool(name="const", bufs=1) as const,
            tc.tile_pool(name="persist", bufs=1) as persist,
        ):
            # ---- constants -------------------------------------------------
            ident_f = const.tile([128, 128], F32)
            make_identity(nc, ident_f[:])
            ident = const.tile([128, 128], F32R)
            nc.vector.tensor_copy(ident[:], ident_f[:])
            ones_fl = const.tile([128, S], F32)
            nc.vector.memset(ones_fl[:], 1.0)
            ones_row = const.tile([1, 128], F32R)
            nc.vector.tensor_copy(ones_row[:], ones_fl[0:1, 0:128])
            ones_col = const.tile([128, 1], F32R)
            nc.vector.tensor_copy(ones_col[:], ones_fl[:, 0:1])

            gam_sb = const.tile([1, H], F32)
            nc.gpsimd.dma_start(out=gam_sb[:], in_=gam_d[None, :])
            # softplus(x) = ln(1 + exp(x)); Softplus has no ACT table here
            g_e = const.tile([1, H], F32)
            nc.scalar.activation(g_e[:], gam_sb[:], AF.Exp)
            g_e1 = const.tile([1, H], F32)
            nc.vector.tensor_scalar_add(g_e1[:], g_e[:], 1.0)
            g_sp = const.tile([1, H], F32)
            nc.scalar.activation(g_sp[:], g_e1[:], AF.Ln)
            ng = const.tile([1, H], F32R)
            nc.vector.tensor_scalar_mul(ng[:], g_sp[:], -1.0)

            bmT = const.tile([128, E // 128], F32)
            nc.gpsimd.dma_start(
                out=bmT[:], in_=bm_d.rearrange("(m p) -> p m", p=128)
            )
            bo_sb = const.tile([1, E], F32R)
            nc.gpsimd.dma_start(out=bo_sb[:], in_=bo_d[None, :])

            # persistent activations (f32r: matmul operands)
            qT_sb = persist.tile([128, 4, S], F32R)   # q^T / 8, [e, s] layout
            kT_sb = persist.tile([128, 4, S], F32R)   # k^T
            v_sb = persist.tile([128, SQ, E], F32R)   # v natural [s, e]
            outT_sb = persist.tile([128, 4, S], F32R)  # per-head outputs^T
            WoT_sb = persist.tile([128, 4, E], F32R)
            nc.sync.dma_start(
                out=WoT_sb[:], in_=WoT_d.rearrange("(c p) e -> p c e", p=128)
            )
            ngbc = persist.tile([128, H], F32)
            colsum_sb = persist.tile([1, E], F32R)

            # ---- phase 1+2: projections -----------------------------------
            with (
                tc.tile_pool(name="ph12", bufs=1) as ph12,
                tc.tile_pool(name="ps12", bufs=3, space="PSUM") as ps12,
                tc.tile_pool(name="ps12s", bufs=2, space="PSUM") as ps12s,
            ):
                combT_sb = ph12.tile([128, 8, S], F32R)
                nc.sync.dma_start(
                    out=combT_sb[:],
                    in_=combT_d.rearrange("(c p) s -> p c s", p=128),
                )
                WmT_sb = ph12.tile([128, 8, E], F32R)
                nc.sync.dma_start(
                    out=WmT_sb[:], in_=WmT_d.rearrange("(c p) e -> p c e", p=128)
                )
                WqT_sb = ph12.tile([128, 4, E], F32R)
                nc.sync.dma_start(
                    out=WqT_sb[:], in_=WqT_d.rearrange("(c p) e -> p c e", p=128)
                )
                WkT_sb = ph12.tile([128, 4, E], F32R)
                nc.sync.dma_start(
                    out=WkT_sb[:], in_=WkT_d.rearrange("(c p) e -> p c e", p=128)
                )
                WvT_sb = ph12.tile([128, 4, E], F32R)
                nc.sync.dma_start(
                    out=WvT_sb[:], in_=WvT_d.rearrange("(c p) e -> p c e", p=128)
                )

                # -g broadcast across partitions via K=1 matmul
                ps_nb = ps12s.tile([128, H], F32, tag="s")
                nc.tensor.matmul(
                    ps_nb[:], ones_row[:], ng[:], start=True, stop=True
                )
                nc.scalar.copy(ngbc[:], ps_nb[:])

                # integrated^T [e, s]
                intT_sb = ph12.tile([128, 4, S], F32R)
                for m in range(4):
                    for sh in range(2):
                        ps_i = ps12.tile([128, 512], F32, tag="mm")
                        for k in range(8):
                            nc.tensor.matmul(
                                ps_i[:],
                                WmT_sb[:, k, m * 128:(m + 1) * 128],
                                combT_sb[:, k, sh * 512:(sh + 1) * 512],
                                start=(k == 0),
                                stop=(k == 7),
                            )
                        nc.scalar.activation(
                            intT_sb[:, m, sh * 512:(sh + 1) * 512],
                            ps_i[:],
                            AF.Identity,
                            bias=bmT[:, m:m + 1],
                        )

                # q^T (scaled by 1/sqrt(D)), k^T
                for (w_sb, dst, scl) in (
                    (WqT_sb, qT_sb, 1.0 / np.sqrt(D)),
                    (WkT_sb, kT_sb, 1.0),
                ):
                    for m in range(4):
                        for sh in range(2):
                            ps_q = ps12.tile([128, 512], F32, tag="mm")
                            for k in range(4):
                                nc.tensor.matmul(
                                    ps_q[:],
                                    w_sb[:, k, m * 128:(m + 1) * 128],
                                    intT_sb[:, k, sh * 512:(sh + 1) * 512],
                                    start=(k == 0),
                                    stop=(k == 3),
                                )
                            if scl != 1.0:
                                nc.scalar.mul(
                                    dst[:, m, sh * 512:(sh + 1) * 512],
                                    ps_q[:], scl,
                                )
                            else:
                                nc.scalar.copy(
                                    dst[:, m, sh * 512:(sh + 1) * 512], ps_q[:]
                                )

                # v natural [s, e]
                for sc in range(SQ):
                    ps_v = ps12.tile([128, 512], F32, tag="mm")
                    for k in range(4):
                        nc.tensor.matmul(
                            ps_v[:],
                            intT_sb[:, k, sc * 128:(sc + 1) * 128],
                            WvT_sb[:, k, :],
                            start=(k == 0),
                            stop=(k == 3),
                        )
                    nc.scalar.copy(v_sb[:, sc, :], ps_v[:])

                # column sums of v (all heads at once): [1, E]
                ps_cs = ps12s.tile([1, E], F32, tag="s")
                for sc in range(SQ):
                    nc.tensor.matmul(
                        ps_cs[:],
                        ones_col[:],
                        v_sb[:, sc, :],
                        start=(sc == 0),
                        stop=(sc == SQ - 1),
                    )
                nc.scalar.copy(colsum_sb[:], ps_cs[:])

            # ---- phase 3: banded attention --------------------------------
            with (
                tc.tile_pool(name="work", bufs=2) as work,
                tc.tile_pool(name="tot_p", bufs=3) as tot_p,
                tc.tile_pool(name="attn_p", bufs=3) as attn_p,
                tc.tile_pool(name="perh", bufs=2) as perh,
                tc.tile_pool(name="ps_s", bufs=2, space="PSUM") as ps_s,
                tc.tile_pool(name="ps_t", bufs=2, space="PSUM") as ps_t,
                tc.tile_pool(name="ps_o", bufs=1, space="PSUM") as ps_o,
                tc.tile_pool(name="ps_r", bufs=2, space="PSUM") as ps_r,
            ):
                # distance tiles, shared across heads (qb-major loop)
                dist_tiles = []
                for qb in range(SQ):
                    ws = _wstart(qb, W)
                    dist_i = work.tile([128, W], F32, tag=f"dist_i{qb % 2}")
                    nc.gpsimd.iota(
                        dist_i[:],
                        pattern=[[-1, W]],
                        base=qb * 128 - ws,
                        channel_multiplier=1,
                        allow_small_or_imprecise_dtypes=True,
                    )
                    dist_a = persist.tile([128, W], F32, tag=f"dist_a{qb}")
                    nc.scalar.activation(dist_a[:], dist_i[:], AF.Abs)
                    dist_tiles.append(dist_a)

                for h in range(8):
                    po, mc = (h % 2) * 64, h // 2
                    # per-head gathered transposed band and r row
                    rtb1T_all = perh.tile([128, SQ, 3 * 128], F32R, tag="bT")
                    rT_all = perh.tile([1, S], F32R, tag="rT")

                    for qb in range(SQ):
                        ws = _wstart(qb, W)
                        # decay = exp(-g_h * dist)
                        decay = work.tile([128, W], F32, tag="decay")
                        nc.scalar.activation(
                            decay[:], dist_tiles[qb][:], AF.Exp,
                            scale=ngbc[:, h:h + 1],
                        )
                        # scores window
                        ps_sc = ps_s.tile([128, W], F32, tag="sc")
                        for j0 in range(0, W, 512):
                            jw = min(512, W - j0)
                            nc.tensor.matmul(
                                ps_sc[:, j0:j0 + jw],
                                qT_sb[po:po + 64, mc, qb * 128:(qb + 1) * 128],
                                kT_sb[po:po + 64, mc, ws + j0:ws + j0 + jw],
                                start=True,
                                stop=True,
                            )
                        # sd = scores * decay;  total = exp(sd), rowsum
                        sd = work.tile([128, W], F32, tag="sd")
                        nc.vector.tensor_mul(sd[:], ps_sc[:], decay[:])
                        tot = tot_p.tile([128, W], F32, tag="tot")
                        rs = work.tile([128, 1], F32, tag="rs")
                        nc.scalar.activation(
                            tot[:], sd[:], AF.Exp, accum_out=rs[:]
                        )
                        rs2 = work.tile([128, 1], F32, tag="rs2")
                        nc.vector.tensor_scalar_add(rs2[:], rs[:], float(S - W))
                        r_t = work.tile([128, 1], F32, tag="r_t")
                        nc.vector.reciprocal(r_t[:], rs2[:])

                        # attn tile: off-window fill r, window total*r
                        at = attn_p.tile([128, S], F32, tag="attn")
                        if ws > 0:
                            nc.vector.tensor_scalar_mul(
                                at[:, 0:ws], ones_fl[:, 0:ws], r_t[:]
                            )
                        if ws + W < S:
                            nc.vector.tensor_scalar_mul(
                                at[:, ws + W:S],
                                ones_fl[:, 0:S - ws - W],
                                r_t[:],
                            )
                        nc.vector.tensor_scalar_mul(
                            at[:, ws:ws + W], tot[:], r_t[:]
                        )
                        nc.sync.dma_start(
                            out=attn_d[h, qb * 128:(qb + 1) * 128, :],
                            in_=at[:],
                        )

                        # rtb1 = (total - 1) * r  (row-scaled band, f32r)
                        rtb1 = work.tile([128, W], F32R, tag="rtb1")
                        nc.vector.tensor_scalar(
                            rtb1[:], tot[:], 1.0, r_t[:],
                            ALU.subtract, ALU.mult,
                        )
                        # r^T row chunk via PE transpose
                        r_r = work.tile([128, 1], F32R, tag="r_r")
                        nc.vector.tensor_copy(r_r[:], r_t[:])
                        ps_rt = ps_r.tile([1, 128], F32R, tag="rt")
                        nc.tensor.transpose(ps_rt[:], r_r[:], ident[:])
                        nc.scalar.copy(
                            rT_all[0:1, qb * 128:(qb + 1) * 128], ps_rt[:]
                        )

                        # transpose band chunks into the per-head gather buf
                        ps_tr = ps_t.tile([128, W], F32R, tag="tr")
                        for c in range(NW):
                            tci = ws // 128 + c
                            slot = qb - tci + half
                            if slot < 0 or slot > 2 * half:
                                continue  # clipped chunk: rtb1 == 0 there
                            nc.tensor.transpose(
                                ps_tr[:, c * 128:(c + 1) * 128],
                                rtb1[:, c * 128:(c + 1) * 128],
                                ident[:],
                            )
                            nc.scalar.copy(
                                rtb1T_all[:, tci, slot * 128:(slot + 1) * 128],
                                ps_tr[:, c * 128:(c + 1) * 128],
                            )

                    # out^T_h[d, q] = colsum ⊗ r^T + sum_tc v_tc^T @ band_tc
                    ps_ov = ps_o.tile([64, S], F32, tag="ov")
                    for j0 in range(0, S, 512):
                        nc.tensor.matmul(
                            ps_ov[:, j0:j0 + 512],
                            colsum_sb[0:1, h * 64:(h + 1) * 64],
                            rT_all[0:1, j0:j0 + 512],
                            start=True,
                            stop=False,
                        )
                    for tci in range(SQ):
                        qlo = max(tci - half, 0)
                        qhi = min(tci + half, SQ - 1)
                        slo = qlo - tci + half
                        # split at PSUM bank boundaries (512 fp32 columns)
                        c0, c1 = qlo * 128, (qhi + 1) * 128
                        segs = []
                        p = c0
                        while p < c1:
                            pe = min(c1, (p // 512 + 1) * 512)
                            segs.append((p, pe))
                            p = pe
                        for si, (p0, p1) in enumerate(segs):
                            nc.tensor.matmul(
                                ps_ov[:, p0:p1],
                                v_sb[:, tci, h * 64:(h + 1) * 64],
                                rtb1T_all[:, tci,
                                          (slo * 128 + p0 - c0):(slo * 128 + p1 - c0)],
                                start=False,
                                stop=(tci == SQ - 1 and si == len(segs) - 1),
                            )
                    nc.scalar.copy(
                        outT_sb[po:po + 64, mc, :], ps_ov[:]
                    )

            # ---- phase 4: output projection -------------------------------
            with (
                tc.tile_pool(name="ph4", bufs=3) as ph4,
                tc.tile_pool(name="ps4", bufs=2, space="PSUM") as ps4,
            ):
                for qb in range(SQ):
                    ps_f = ps4.tile([128, E], F32)
                    for ec in range(4):
                        nc.tensor.matmul(
                            ps_f[:],
                            outT_sb[:, ec, qb * 128:(qb + 1) * 128],
                            WoT_sb[:, ec, :],
                            start=(ec == 0),
                            stop=False,
                        )
                    nc.tensor.matmul(
                        ps_f[:], ones_row[:], bo_sb[:], start=False, stop=True
                    )
                    o_sb = ph4.tile([128, E], F32, tag="o")
                    nc.scalar.copy(o_sb[:], ps_f[:])
                    nc.sync.dma_start(
                        out=out_d[qb * 128:(qb + 1) * 128, :], in_=o_sb[:]
                    )

    nc.compile()
    return nc


def kernel(**inputs):
    x = np.asarray(inputs["unified_embed"], dtype=np.float32)
    mem = np.asarray(inputs["memory_state"], dtype=np.float32)
    Wq = np.asarray(inputs["Wq"], dtype=np.float32)
    Wk = np.asarray(inputs["Wk"], dtype=np.float32)
    Wv = np.asarray(inputs["Wv"], dtype=np.float32)
    Wm = np.asarray(inputs["Wm"], dtype=np.float32)
    bm = np.asarray(inputs["bm"], dtype=np.float32)
    Wo = np.asarray(inputs["Wo"], dtype=np.float32)
    bo = np.asarray(inputs["bo"], dtype=np.float32)
    gammas = np.asarray(inputs["gammas"], dtype=np.float32)

    g = np.logaddexp(0.0, gammas.astype(np.float64))  # softplus, host-side
    W = _pick_window(float(g.min()))

    if W not in _CACHE:
        _CACHE[W] = _build(W)
    nc = _CACHE[W]

    WmT = np.ascontiguousarray(Wm.T)
    WqT = np.ascontiguousarray(Wq.T)
    WkT = np.ascontiguousarray(Wk.T)
    WvT = np.ascontiguousarray(Wv.T)
    WoT = np.ascontiguousarray(Wo.T)

    in_maps = []
    for b in range(NCORES):
        combT = np.ascontiguousarray(
            np.concatenate([x[b], mem[b]], axis=1).T
        )
        in_maps.append({
            "combT": combT,
            "WmT": WmT, "WqT": WqT, "WkT": WkT, "WvT": WvT, "WoT": WoT,
            "bm": bm, "bo": bo, "gam": gammas,
        })

    res = run_bass_kernel_spmd(
        nc, in_maps, core_ids=list(range(NCORES)),
        trace=bool(int(os.environ.get("KERNEL_TRACE", "0"))),
    )
    LAST_PERF["exec_time_ns"] = res.exec_time_ns
    LAST_PERF["mean_exec_time_ns"] = res.mean_exec_time_ns
    LAST_PERF["trace"] = res.instructions_and_trace

    out = np.stack([res.results[b]["out"] for b in range(NCORES)])
    attn = np.stack([res.results[b]["attn"] for b in range(NCORES)])
    return out, attn


# revision 9
# speedup vs baseline: 1.8952x; 1.4109x over previous
"""Trainium2 Bass kernel for nn_MemoryAwareAKTAttention.

Math (per batch b):
    integrated = concat([x, mem], -1) @ Wm.T + bm          [S, E]
    q, k, v    = heads(integrated @ W{q,k,v}.T)            [H, S, D]
    scores     = q @ k.T / sqrt(D)                         [H, S, S]
    decay      = exp(-softplus(gamma_h) * |i-j|)           [H, S, S]
    total      = clip(exp(scores * decay), 1e-5, 1e5)
    attn       = total / (sum_j total + 1e-9)
    out        = (attn @ v) @ Wo.T + bo                    [S, E]

Structure exploited: with g = softplus(gamma) >= ~0.17, decay underflows to
0.0 within ~129 columns of the diagonal, so total == 1.0f bit-exactly outside
a narrow band (exp(x) rounds to 1.0f for |x| < 2^-25).  Per 128-row query
block we keep a persistent [128, S] total tile whose off-window region is
memset to 1.0 once; only the W-wide window is recomputed per head.  attn is
then a single row-scale of that tile.  For out = attn @ v, the off-band part
folds into a per-head column-sum of v (rank-1 colsum x r^T), and the row sums
fall out of the banded matmul itself via a ones-column appended to v:
sum_j (total-1)*r = 1 - S*r.

Matmuls run in float32r (fp32 runs at 4 cycles/row; f32r at 2); the
exp/normalization chain stays fp32.

Sharding: data-parallel over batch, one batch element per NeuronCore (B=8).
"""

import os
import numpy as np

import concourse.bass as bass
import concourse.mybir as mybir
import concourse.tile as tile
from concourse import bacc
from concourse.bass_utils import run_bass_kernel_spmd
from concourse.masks import make_identity

F32 = mybir.dt.float32
F32R = mybir.dt.float32r
AF = mybir.ActivationFunctionType
ALU = mybir.AluOpType

B, S, E = 8, 1024, 512
H = 8
D = E // H          # 64
TWO_E = 2 * E
NCORES = 8
SQ = S // 128       # query blocks per core

LAST_PERF = {}      # filled by kernel(): exec_time_ns etc.
_CACHE = {}         # W -> compiled Bacc module


def _wstart(qb: int, W: int) -> int:
    """Window start for query block qb; multiple of 128, centered, clipped."""
    n = W // 128
    return min(max((qb - (n - 1) // 2) * 128, 0), S - W)


def _pick_window(g_min: float) -> int:
    """Smallest window width such that off-window total == 1.0f exactly.

    Off-window distance >= margin(W); need exp(-g*margin) * SMAX < 2^-25
    with SMAX = 64 a generous bound on |scores|.  ln(64 * 2^26) ~= 22.18.
    """
    for n in (3, 5, 7):
        W = n * 128
        margin = ((n - 1) // 2 - 1) * 128 + 129
        if g_min * margin >= 22.18:
            return W
    return S  # dense fallback: every column computed exactly


def _build(W: int) -> bacc.Bacc:
    nc = bacc.Bacc(None)
    NW = W // 128
    half = (NW - 1) // 2
    NSLOT = 2 * half + 1  # transposed-band slots per t-chunk

    combT_d = nc.dram_tensor("combT", [TWO_E, S], F32R, kind="ExternalInput")
    WmT_d = nc.dram_tensor("WmT", [TWO_E, E], F32R, kind="ExternalInput")
    WqT_d = nc.dram_tensor("WqT", [E, E], F32R, kind="ExternalInput")
    WkT_d = nc.dram_tensor("WkT", [E, E], F32R, kind="ExternalInput")
    WvT_d = nc.dram_tensor("WvT", [E, E], F32R, kind="ExternalInput")
    WoT_d = nc.dram_tensor("WoT", [E, E], F32R, kind="ExternalInput")
    bm_d = nc.dram_tensor("bm", [E], F32, kind="ExternalInput")
    bo_d = nc.dram_tensor("bo", [E], F32R, kind="ExternalInput")
    gam_d = nc.dram_tensor("gam", [H], F32, kind="ExternalInput")
    out_d = nc.dram_tensor("out", [S, E], F32, kind="ExternalOutput")
    attn_d = nc.dram_tensor("attn", [H, S, S], F32, kind="ExternalOutput")

    with tile.TileContext(nc) as tc:
        with (
            tc.tile_pool(name="const", bufs=1) as const,
            tc.tile_pool(name="persist", bufs=1) as persist,
        ):
            # ---- constants -------------------------------------------------
            ident_f = const.tile([128, 128], F32)
            make_identity(nc, ident_f[:])
            ident = const.tile([128, 128], F32R)
            nc.vector.tensor_copy(ident[:], ident_f[:])
            ones_fl = const.tile([128, S], F32)
            nc.vector.memset(ones_fl[:], 1.0)
            ones_row = const.tile([1, 128], F32R)
            nc.vector.tensor_copy(ones_row[:], ones_fl[0:1, 0:128])
            zeros_f = const.tile([128, 65], F32)
            nc.vector.memset(zeros_f[:], 0.0)
            zeros_a = const.tile([1, 65], F32R)
            nc.vector.tensor_copy(zeros_a[:], zeros_f[0:1, :])
            zrow = const.tile([1, 512], F32R)
            nc.vector.tensor_copy(zrow[:], ones_fl[0:1, 0:512])
            ones_col = const.tile([128, 1], F32R)
            nc.vector.tensor_copy(ones_col[:], ones_fl[:, 0:1])

            gam_sb = const.tile([1, H], F32)
            nc.gpsimd.dma_start(out=gam_sb[:], in_=gam_d[None, :])
            # softplus(x) = ln(1 + exp(x)); Softplus has no ACT table here
            g_e = const.tile([1, H], F32)
            nc.scalar.activation(g_e[:], gam_sb[:], AF.Exp)
            g_e1 = const.tile([1, H], F32)
            nc.vector.tensor_scalar_add(g_e1[:], g_e[:], 1.0)
            g_sp = const.tile([1, H], F32)
            nc.scalar.activation(g_sp[:], g_e1[:], AF.Ln)
            ng = const.tile([1, H], F32R)
            nc.vector.tensor_scalar_mul(ng[:], g_sp[:], -1.0)

            bmT = const.tile([128, E // 128], F32)
            nc.gpsimd.dma_start(
                out=bmT[:], in_=bm_d.rearrange("(m p) -> p m", p=128)
            )
            bo_sb = const.tile([1, E], F32R)
            nc.gpsimd.dma_start(out=bo_sb[:], in_=bo_d[None, :])

            # persistent activations (f32r: matmul operands)
            qT_sb = persist.tile([128, 4, S], F32R)    # q^T / 8, [e, s]
            kT_sb = persist.tile([128, 4, S], F32R)    # k^T
            # v augmented with a ones column per head: [s, (h, d|1)]
            v_aug = persist.tile([128, SQ, H, D + 1], F32R)
            outT_sb = persist.tile([128, 4, S], F32R)  # per-head outputs^T
            WoT_sb = persist.tile([128, 4, E], F32R)
            nc.sync.dma_start(
                out=WoT_sb[:], in_=WoT_d.rearrange("(c p) e -> p c e", p=128)
            )
            ngbc = persist.tile([128, H], F32)
            colsum_sb = persist.tile([1, E], F32R)

            # ---- phase 1+2: projections -----------------------------------
            with (
                tc.tile_pool(name="ph12", bufs=1) as ph12,
                tc.tile_pool(name="ps12", bufs=3, space="PSUM") as ps12,
                tc.tile_pool(name="ps12s", bufs=2, space="PSUM") as ps12s,
            ):
                combT_sb = ph12.tile([128, 8, S], F32R)
                nc.sync.dma_start(
                    out=combT_sb[:],
                    in_=combT_d.rearrange("(c p) s -> p c s", p=128),
                )
                WmT_sb = ph12.tile([128, 8, E], F32R)
                nc.sync.dma_start(
                    out=WmT_sb[:], in_=WmT_d.rearrange("(c p) e -> p c e", p=128)
                )
                WqT_sb = ph12.tile([128, 4, E], F32R)
                nc.sync.dma_start(
                    out=WqT_sb[:], in_=WqT_d.rearrange("(c p) e -> p c e", p=128)
                )
                WkT_sb = ph12.tile([128, 4, E], F32R)
                nc.sync.dma_start(
                    out=WkT_sb[:], in_=WkT_d.rearrange("(c p) e -> p c e", p=128)
                )
                WvT_sb = ph12.tile([128, 4, E], F32R)
                nc.sync.dma_start(
                    out=WvT_sb[:], in_=WvT_d.rearrange("(c p) e -> p c e", p=128)
                )

                # -g broadcast across partitions via K=1 matmul
                ps_nb = ps12s.tile([128, H], F32, tag="s")
                nc.tensor.matmul(
                    ps_nb[:], ones_row[:], ng[:], start=True, stop=True
                )
                nc.scalar.copy(ngbc[:], ps_nb[:])

                # integrated^T [e, s]
                intT_sb = ph12.tile([128, 4, S], F32R)
                for m in range(4):
                    for sh in range(2):
                        ps_i = ps12.tile([128, 512], F32, tag="mm")
                        for k in range(8):
                            nc.tensor.matmul(
                                ps_i[:],
                                WmT_sb[:, k, m * 128:(m + 1) * 128],
                                combT_sb[:, k, sh * 512:(sh + 1) * 512],
                                start=(k == 0),
                                stop=(k == 7),
                            )
                        nc.scalar.activation(
                            intT_sb[:, m, sh * 512:(sh + 1) * 512],
                            ps_i[:],
                            AF.Identity,
                            bias=bmT[:, m:m + 1],
                        )

                # q^T (scaled by 1/sqrt(D)), k^T
                for (w_sb, dst, scl) in (
                    (WqT_sb, qT_sb, 1.0 / np.sqrt(D)),
                    (WkT_sb, kT_sb, 1.0),
                ):
                    for m in range(4):
                        for sh in range(2):
                            ps_q = ps12.tile([128, 512], F32, tag="mm")
                            for k in range(4):
                                nc.tensor.matmul(
                                    ps_q[:],
                                    w_sb[:, k, m * 128:(m + 1) * 128],
                                    intT_sb[:, k, sh * 512:(sh + 1) * 512],
                                    start=(k == 0),
                                    stop=(k == 3),
                                )
                            if scl != 1.0:
                                nc.scalar.mul(
                                    dst[:, m, sh * 512:(sh + 1) * 512],
                                    ps_q[:], scl,
                                )
                            else:
                                nc.scalar.copy(
                                    dst[:, m, sh * 512:(sh + 1) * 512], ps_q[:]
                                )

                # v natural [s, e] scattered into the ones-augmented layout
                for sc in range(SQ):
                    ps_v = ps12.tile([128, 512], F32, tag="mm")
                    for k in range(4):
                        nc.tensor.matmul(
                            ps_v[:],
                            intT_sb[:, k, sc * 128:(sc + 1) * 128],
                            WvT_sb[:, k, :],
                            start=(k == 0),
                            stop=(k == 3),
                        )
                    nc.scalar.copy(v_aug[:, sc, :, 0:D], ps_v[:])
                    nc.vector.tensor_copy(v_aug[:, sc, :, D], ones_fl[:, 0:H])

                # column sums of v (all heads at once): [1, E]
                ps_cs = ps12s.tile([1, E], F32, tag="s")
                for sc in range(SQ):
                    nc.tensor.matmul(
                        ps_cs[:],
                        ones_col[:],
                        v_aug[:, sc, :, 0:D],
                        start=(sc == 0),
                        stop=(sc == SQ - 1),
                    )
                nc.scalar.copy(colsum_sb[:], ps_cs[:])

            # ---- phase 3: banded attention --------------------------------
            with (
                tc.tile_pool(name="blk", bufs=1) as blk,
                tc.tile_pool(name="work", bufs=3) as work,
                tc.tile_pool(name="attn_p", bufs=3) as attn_p,
                tc.tile_pool(name="perh", bufs=2) as perh,
                tc.tile_pool(name="ps_s", bufs=3, space="PSUM") as ps_s,
                tc.tile_pool(name="ps_t", bufs=3, space="PSUM") as ps_t,
                tc.tile_pool(name="ps_o", bufs=1, space="PSUM") as ps_o,
            ):
                # per-block persistent tiles: |i-j| distances and total,
                # whose off-window region is 1.0 forever.
                dist_tiles, tot_tiles = [], []
                for qb in range(SQ):
                    ws = _wstart(qb, W)
                    dist_i = work.tile([128, W], F32, tag="dist_i")
                    nc.gpsimd.iota(
                        dist_i[:],
                        pattern=[[-1, W]],
                        base=qb * 128 - ws,
                        channel_multiplier=1,
                        allow_small_or_imprecise_dtypes=True,
                    )
                    dist_a = blk.tile([128, W], F32, tag=f"dist_a{qb}")
                    nc.scalar.activation(dist_a[:], dist_i[:], AF.Abs)
                    dist_tiles.append(dist_a)
                    tot = blk.tile([128, S], F32, tag=f"tot{qb}")
                    if ws > 0:
                        nc.vector.memset(tot[:, 0:ws], 1.0)
                    if ws + W < S:
                        nc.vector.memset(tot[:, ws + W:S], 1.0)
                    tot_tiles.append(tot)

                for h in range(8):
                    po, mc = (h % 2) * 64, h // 2
                    # per-head gathered transposed band and r row
                    rtb1T_all = perh.tile([128, SQ, NSLOT * 128], F32R, tag="bT")
                    rT_all = perh.tile([1, S], F32R, tag="rT")

                    for qb in range(SQ):
                        ws = _wstart(qb, W)
                        tot = tot_tiles[qb]
                        # decay = exp(-g_h * dist)
                        decay = work.tile([128, W], F32, tag="decay")
                        nc.scalar.activation(
                            decay[:], dist_tiles[qb][:], AF.Exp,
                            scale=ngbc[:, h:h + 1],
                        )
                        # scores window
                        ps_sc = ps_s.tile([128, W], F32, tag="sc")
                        for j0 in range(0, W, 512):
                            jw = min(512, W - j0)
                            nc.tensor.matmul(
                                ps_sc[:, j0:j0 + jw],
                                qT_sb[po:po + 64, mc, qb * 128:(qb + 1) * 128],
                                kT_sb[po:po + 64, mc, ws + j0:ws + j0 + jw],
                                start=True,
                                stop=True,
                            )
                        # sd = scores * decay;  total = exp(sd), band rowsum
                        sd = work.tile([128, W], F32, tag="sd")
                        nc.vector.tensor_mul(sd[:], ps_sc[:], decay[:])
                        rs = work.tile([128, 1], F32, tag="rs")
                        nc.scalar.activation(
                            tot[:, ws:ws + W], sd[:], AF.Exp, accum_out=rs[:]
                        )
                        rs2 = work.tile([128, 1], F32, tag="rs2")
                        nc.vector.tensor_scalar_add(rs2[:], rs[:], float(S - W))
                        r_t = work.tile([128, 1], F32, tag="r_t")
                        nc.vector.reciprocal(r_t[:], rs2[:])

                        # attn tile = total * r in one pass (flanks are 1.0)
                        at = attn_p.tile([128, S], F32, tag="attn")
                        nc.vector.tensor_scalar_mul(at[:], tot[:], r_t[:])
                        nc.sync.dma_start(
                            out=attn_d[h, qb * 128:(qb + 1) * 128, :],
                            in_=at[:],
                        )

                        # rtb1 = (total - 1) * r  (row-scaled band, f32r)
                        rtb1 = work.tile([128, W], F32R, tag="rtb1")
                        nc.vector.tensor_scalar(
                            rtb1[:], tot[:, ws:ws + W], 1.0, r_t[:],
                            ALU.subtract, ALU.mult,
                        )
                        # transpose band chunks, gather into per-head buffer
                        ps_tr = ps_t.tile([128, W], F32R, tag="tr")
                        c_lo, c_hi = None, None
                        for c in range(NW):
                            tci = ws // 128 + c
                            slot = qb - tci + half
                            if slot < 0 or slot >= NSLOT:
                                continue
                            nc.tensor.transpose(
                                ps_tr[:, c * 128:(c + 1) * 128],
                                rtb1[:, c * 128:(c + 1) * 128],
                                ident[:],
                            )
                            if c_lo is None:
                                c_lo = c
                            c_hi = c
                        # one strided copy: consecutive chunks land
                        # (NSLOT-1)*128 apart in the gather buffer
                        tci0 = ws // 128 + c_lo
                        slot0 = qb - tci0 + half
                        nch = c_hi - c_lo + 1
                        full = rtb1T_all[:]
                        dst = bass.AP(
                            tensor=full.tensor,
                            offset=full.offset + tci0 * (NSLOT * 128) + slot0 * 128,
                            ap=[full.ap[0],
                                [(NSLOT - 1) * 128, nch],
                                [1, 128]],
                        )
                        nc.scalar.copy(
                            dst, ps_tr[:, c_lo * 128:(c_hi + 1) * 128]
                        )

                    # out^T_h[d,q] (+ row 64 = band sums) over all q at once
                    ps_ov = ps_o.tile([65, S], F32, tag="ov")
                    for j0 in range(0, S, 512):
                        nc.tensor.matmul(
                            ps_ov[:, j0:j0 + 512],
                            zeros_a[:],
                            zrow[:],
                            start=True,
                            stop=False,
                        )
                    for tci in range(SQ):
                        qlo = max(tci - half, 0)
                        qhi = min(tci + half, SQ - 1)
                        slo = qlo - tci + half
                        c0, c1 = qlo * 128, (qhi + 1) * 128
                        p = c0
                        while p < c1:
                            pe = min(c1, (p // 512 + 1) * 512)
                            nc.tensor.matmul(
                                ps_ov[:, p:pe],
                                v_aug[:, tci, h, :],
                                rtb1T_all[:, tci,
                                          (slo * 128 + p - c0):(slo * 128 + pe - c0)],
                                start=False,
                                stop=False,
                            )
                            p = pe
                    # r^T row = (1 - band_sum) / S, from psum row 64
                    brow = work.tile([1, S], F32, tag="brow")
                    nc.scalar.copy(brow[:], ps_ov[64:65, :])
                    nc.vector.tensor_scalar(
                        rT_all[:], brow[:], 1.0, -1.0 / S,
                        ALU.subtract, ALU.mult,
                    )
                    # off-band term: colsum ⊗ r^T accumulated on top
                    for j0 in range(0, S, 512):
                        nc.tensor.matmul(
                            ps_ov[0:64, j0:j0 + 512],
                            colsum_sb[0:1, h * 64:(h + 1) * 64],
                            rT_all[0:1, j0:j0 + 512],
                            start=False,
                            stop=(j0 == 512),
                        )
                    nc.scalar.copy(
                        outT_sb[po:po + 64, mc, :], ps_ov[0:64, :]
                    )

            # ---- phase 4: output projection -------------------------------
            with (
                tc.tile_pool(name="ph4", bufs=3) as ph4,
                tc.tile_pool(name="ps4", bufs=2, space="PSUM") as ps4,
            ):
                for qb in range(SQ):
                    ps_f = ps4.tile([128, E], F32)
                    for ec in range(4):
                        nc.tensor.matmul(
                            ps_f[:],
                            outT_sb[:, ec, qb * 128:(qb + 1) * 128],
                            WoT_sb[:, ec, :],
                            start=(ec == 0),
                            stop=False,
                        )
                    nc.tensor.matmul(
                        ps_f[:], ones_row[:], bo_sb[:], start=False, stop=True
                    )
                    o_sb = ph4.tile([128, E], F32, tag="o")
                    nc.scalar.copy(o_sb[:], ps_f[:])
                    nc.sync.dma_start(
                        out=out_d[qb * 128:(qb + 1) * 128, :], in_=o_sb[:]
                    )

    nc.compile()
    return nc


def kernel(**inputs):
    x = np.asarray(inputs["unified_embed"], dtype=np.float32)
    mem = np.asarray(inputs["memory_state"], dtype=np.float32)
    Wq = np.asarray(inputs["Wq"], dtype=np.float32)
    Wk = np.asarray(inputs["Wk"], dtype=np.float32)
    Wv = np.asarray(inputs["Wv"], dtype=np.float32)
    Wm = np.asarray(inputs["Wm"], dtype=np.float32)
    bm = np.asarray(inputs["bm"], dtype=np.float32)
    Wo = np.asarray(inputs["Wo"], dtype=np.float32)
    bo = np.asarray(inputs["bo"], dtype=np.float32)
    gammas = np.asarray(inputs["gammas"], dtype=np.float32)

    g = np.logaddexp(0.0, gammas.astype(np.float64))  # softplus, host-side
    W = _pick_window(float(g.min()))

    if W not in _CACHE:
        _CACHE[W] = _build(W)
    nc = _CACHE[W]

    WmT = np.ascontiguousarray(Wm.T)
    WqT = np.ascontiguousarray(Wq.T)
    WkT = np.ascontiguousarray(Wk.T)
    WvT = np.ascontiguousarray(Wv.T)
    WoT = np.ascontiguousarray(Wo.T)

    in_maps = []
    for b in range(NCORES):
        combT = np.ascontiguousarray(
            np.concatenate([x[b], mem[b]], axis=1).T
        )
        in_maps.append({
            "combT": combT,
            "WmT": WmT, "WqT": WqT, "WkT": WkT, "WvT": WvT, "WoT": WoT,
            "bm": bm, "bo": bo, "gam": gammas,
        })

    res = run_bass_kernel_spmd(
        nc, in_maps, core_ids=list(range(NCORES)),
        trace=bool(int(os.environ.get("KERNEL_TRACE", "0"))),
    )
    LAST_PERF["exec_time_ns"] = res.exec_time_ns
    LAST_PERF["mean_exec_time_ns"] = res.mean_exec_time_ns
    LAST_PERF["trace"] = res.instructions_and_trace

    out = np.stack([res.results[b]["out"] for b in range(NCORES)])
    attn = np.stack([res.results[b]["attn"] for b in range(NCORES)])
    return out, attn
